# revision 1
# baseline (speedup 1.0000x reference)
"""Trainium2 Bass kernel for nn_Block_6975026889258 (gnn_message_passing).

Distribution: nodes (rows of x / adj / M) are sharded across 8 NeuronCores.
Whitened features Z are replicated to every core (the "all-gather" is done by
feeding every core the full Z operand). The two N x N @ N x d products and the
N x N pairwise-score matrix + top-k run row-sharded on device; the tiny
O(N*d^2) graph assembly (Cholesky of the d x d Gram matrix, sparse scatter of
16 entries/row, degree normalization) runs on host between the device stages.
"""
import numpy as np

import concourse.bacc as bacc
import concourse.mybir as mybir
from concourse.tile import TileContext
from concourse.bass_utils import run_bass_kernel_spmd

N = 8192
D_IN = 512
D_OUT = 256
K = 16
ALPHA = 0.5
BETA = 1.0
NCORES = 8
R = N // NCORES          # 1024 rows per core
P = 128                  # partitions
RT = R // P              # row tiles per core (8)
JC = N // 512            # 512-wide column chunks (16)

f32 = mybir.dt.float32
f32r = mybir.dt.float32r
u32 = mybir.dt.uint32

_programs = {}


def _build_score_topk(dz):
    """Program: s = UT.T @ VT ([R,dz] @ [dz,N] scores), top-16 (vals+idx) per row.

    Inputs: UT [dz, R] f32 (lhsT, local rows' whitened feats + ones row),
            VT [dz, N] f32 (replicated [Zt; -sq/2]).
    Outputs: val16 [R, 16] f32 (score s, descending), idx16 [R, 16] u32.
    """
    nc = bacc.Bacc("TRN2", num_devices=NCORES)
    ut_d = nc.dram_tensor("UT", [dz, R], f32, kind="ExternalInput")
    vt_d = nc.dram_tensor("VT", [dz, N], f32, kind="ExternalInput")
    val_d = nc.dram_tensor("VAL16", [R, 32], f32, kind="ExternalOutput")
    idx_d = nc.dram_tensor("IDX16", [R, 32], u32, kind="ExternalOutput")

    nkb = (dz + P - 1) // P  # k-blocks (last one partial)
    NH = N // 2

    with TileContext(nc) as tc:
        with tc.tile_pool(name="vt", bufs=1) as vpool, \
             tc.tile_pool(name="ut", bufs=1) as upool, \
             tc.tile_pool(name="work", bufs=1) as wpool, \
             tc.tile_pool(name="small", bufs=2) as spool, \
             tc.tile_pool(name="ps", bufs=4, space="PSUM") as psp:

            uts = []
            for kb in range(nkb):
                kp = min(P, dz - kb * P)
                ut = upool.tile([P, R], f32r, tag=f"ut{kb}")
                nc.sync.dma_start(out=ut[:kp, :],
                                  in_=ut_d[kb * P:kb * P + kp, :].bitcast(f32r))
                uts.append((ut, kp))

            for half in range(2):
                vts = []
                for kb in range(nkb):
                    kp = min(P, dz - kb * P)
                    vt = vpool.tile([P, NH], f32r, tag=f"vt{kb}")
                    for c in range(2):
                        nc.sync.dma_start(
                            out=vt[:kp, c * 2048:(c + 1) * 2048],
                            in_=vt_d[kb * P:kb * P + kp,
                                     half * NH + c * 2048:half * NH + (c + 1) * 2048
                                     ].bitcast(f32r))
                    vts.append((vt, kp))

                for rt in range(RT):
                    s_sb = wpool.tile([P, NH], f32, tag="s_sb")
                    for jc in range(NH // 512):
                        ps = psp.tile([P, 512], f32, tag="ps")
                        for kb in range(nkb):
                            ut, kp = uts[kb]
                            vt, _ = vts[kb]
                            nc.tensor.matmul(
                                out=ps,
                                lhsT=ut[:kp, rt * P:(rt + 1) * P],
                                rhs=vt[:kp, jc * 512:(jc + 1) * 512],
                                start=(kb == 0), stop=(kb == nkb - 1))
                        nc.scalar.copy(out=s_sb[:, jc * 512:(jc + 1) * 512], in_=ps)

                    v1 = spool.tile([P, 8], f32, tag="v1")
                    i1 = spool.tile([P, 8], u32, tag="i1")
                    v2 = spool.tile([P, 8], f32, tag="v2")
                    i2 = spool.tile([P, 8], u32, tag="i2")
                    nc.vector.max(out=v1, in_=s_sb)
                    nc.vector.max_index(out=i1, in_max=v1, in_values=s_sb)
                    nc.vector.match_replace(out=s_sb, in_to_replace=v1,
                                            in_values=s_sb, imm_value=-3e38)
                    nc.vector.max(out=v2, in_=s_sb)
                    nc.vector.max_index(out=i2, in_max=v2, in_values=s_sb)
                    vcat = spool.tile([P, 16], f32, tag="vcat")
                    icat = spool.tile([P, 16], u32, tag="icat")
                    nc.vector.tensor_copy(out=vcat[:, 0:8], in_=v1)
                    nc.vector.tensor_copy(out=vcat[:, 8:16], in_=v2)
                    nc.vector.tensor_copy(out=icat[:, 0:8], in_=i1)
                    nc.vector.tensor_copy(out=icat[:, 8:16], in_=i2)
                    nc.sync.dma_start(
                        out=val_d[rt * P:(rt + 1) * P, half * 16:half * 16 + 16],
                        in_=vcat)
                    nc.sync.dma_start(
                        out=idx_d[rt * P:(rt + 1) * P, half * 16:half * 16 + 16],
                        in_=icat)

    nc.compile()
    return nc


def _build_product():
    """Program: OUT = tanh(AT.T @ H) for the local row block.

    Inputs: AT [N, R] f32 (newadj[rows_loc, :]^T column-slab),
            H [N, F] f32 (replicated dense features).
    Output: OUT [R, F] f32 rows of tanh(newadj @ H).
    """
    nc = bacc.Bacc("TRN2", num_devices=NCORES)
    at_d = nc.dram_tensor("AT", [N, R], f32, kind="ExternalInput")
    h_d = nc.dram_tensor("H", [N, D_OUT], f32, kind="ExternalInput")
    out_d = nc.dram_tensor("OUT", [R, D_OUT], f32, kind="ExternalOutput")

    with TileContext(nc) as tc:
        with tc.tile_pool(name="h", bufs=1) as hpool, \
             tc.tile_pool(name="at", bufs=4) as apool, \
             tc.tile_pool(name="o", bufs=2) as opool, \
             tc.tile_pool(name="ps", bufs=4, space="PSUM") as psp:

            hs = []
            for kb in range(N // P):  # 64 j-chunks
                h = hpool.tile([P, D_OUT], f32, tag=f"h{kb}")
                nc.sync.dma_start(out=h, in_=h_d[kb * P:(kb + 1) * P, :])
                hs.append(h)

            for rt in range(RT):
                ps = psp.tile([P, D_OUT], f32, tag="ps")
                for kb in range(N // P):
                    at = apool.tile([P, P], f32, tag="at")
                    nc.sync.dma_start(
                        out=at,
                        in_=at_d[kb * P:(kb + 1) * P, rt * P:(rt + 1) * P])
                    nc.tensor.matmul(out=ps, lhsT=at, rhs=hs[kb],
                                     start=(kb == 0), stop=(kb == N // P - 1))
                o = opool.tile([P, D_OUT], f32, tag="o")
                nc.scalar.activation(out=o, in_=ps,
                                     func=mybir.ActivationFunctionType.Tanh)
                nc.sync.dma_start(out=out_d[rt * P:(rt + 1) * P, :], in_=o)

    nc.compile()
    return nc


def _run(nc, in_maps):
    res = run_bass_kernel_spmd(nc, in_maps, core_ids=list(range(NCORES)))
    return res.results


def _rescore(Zt, sq, idxs):
    """Host: exact d2 for the 32 device candidates, pick 16 smallest
    (lowest index on ties, matching jax top_k), in fp32 like the reference."""
    idxs = idxs.astype(np.int64)
    idxs[:, 16:] += N // 2
    Z = Zt.T                                   # [N, d] f32
    g = Z[idxs]                                # [N, 32, d]
    zz = np.einsum("nd,nkd->nk", Z, g, dtype=np.float64)
    d2 = sq[:, None].astype(np.float64) + sq[idxs].astype(np.float64) - 2.0 * zz
    order = np.lexsort((idxs, d2), axis=1)[:, :K]
    return (np.take_along_axis(d2, order, 1).astype(np.float32),
            np.take_along_axis(idxs, order, 1))


def _graph_from_topk(d2, idxs, sq):
    """Host: reference get_M tail from exact candidate distances."""
    d2 = np.clip(d2, 0.0, None)
    valsr = np.sqrt(d2)
    sigma = valsr.mean()
    kern = np.exp(-d2 / (2.0 * sigma * sigma)).astype(np.float32)
    M = np.zeros((N, N), np.float32)
    np.add.at(M, (np.repeat(np.arange(N), K), idxs.reshape(-1).astype(np.int64)),
              kern.reshape(-1))
    M = 0.5 * (M + M.T)
    deg = M.sum(1)
    dis = np.where(deg > 0, deg ** -0.5, 0.0).astype(np.float32)
    return dis[:, None] * M * dis[None, :]


def _stage_scores(H, beta):
    """Host prep for the device score+topk launch. Returns (UT slabs, VT)."""
    d = H.shape[1]
    import scipy.linalg as sla
    Hf = H.astype(np.float32)
    A = beta * np.eye(d, dtype=np.float32) + Hf.T @ Hf
    L = np.linalg.cholesky(A)
    Zt = sla.solve_triangular(L, Hf.T, lower=True).astype(np.float32)  # [d, N]
    sq = (Zt.astype(np.float64) ** 2).sum(0).astype(np.float32)  # [N]
    VT = np.concatenate([Zt, (-0.5 * sq)[None, :]], 0).astype(np.float32)  # [d+1, N]
    ones = np.ones((1, R), np.float32)
    UTs = [np.ascontiguousarray(
        np.concatenate([Zt[:, p * R:(p + 1) * R], ones], 0)) for p in range(NCORES)]
    return UTs, VT, sq


def kernel(x, adj, weight1, weight2):
    x = np.asarray(x, np.float32)
    adj = np.asarray(adj, np.float32)
    w1 = np.asarray(weight1, np.float32)
    w2 = np.asarray(weight2, np.float32)

    if "score513" not in _programs:
        _programs["score513"] = _build_score_topk(D_IN + 1)
        _programs["score257"] = _build_score_topk(D_OUT + 1)
        _programs["product"] = _build_product()

    # ---------------- stage 1 ----------------
    UTs, VT, sq = _stage_scores(x, BETA)
    res = _run(_programs["score513"],
               [dict(UT=UTs[p], VT=VT) for p in range(NCORES)])
    vals = np.concatenate([r["VAL16"] for r in res], 0)
    idxs = np.concatenate([r["IDX16"] for r in res], 0)
    d2c, idxs = _rescore(VT[:-1], sq, idxs)
    S1 = _graph_from_topk(d2c, idxs, sq)
    newadj1 = ALPHA * adj + S1
    H1 = x @ w1                                            # [N, F]
    res = _run(_programs["product"],
               [dict(AT=np.ascontiguousarray(newadj1[p * R:(p + 1) * R, :].T),
                     H=H1) for p in range(NCORES)])
    out1 = np.concatenate([r["OUT"] for r in res], 0)      # [N, F]

    # ---------------- stage 2 ----------------
    UTs, VT, sq = _stage_scores(out1, BETA)
    res = _run(_programs["score257"],
               [dict(UT=UTs[p], VT=VT) for p in range(NCORES)])
    vals = np.concatenate([r["VAL16"] for r in res], 0)
    idxs = np.concatenate([r["IDX16"] for r in res], 0)
    d2c, idxs = _rescore(VT[:-1], sq, idxs)
    S2 = _graph_from_topk(d2c, idxs, sq)
    newadj2 = ALPHA * adj + S2
    w2s = 0.5 * (w2 + w2.T)
    H2 = out1 @ w2s
    res = _run(_programs["product"],
               [dict(AT=np.ascontiguousarray(newadj2[p * R:(p + 1) * R, :].T),
                     H=H2) for p in range(NCORES)])
    out2 = np.concatenate([r["OUT"] for r in res], 0)
    return out2

